# revision 2
# baseline (speedup 1.0000x reference)
"""Trainium2 Bass kernel for the e3nn-style 3D convolution problem.

Host side: builds the tiny [3,3,3,32,64] conv kernel from the radial/spherical
weights (replicating the reference math in fp32 numpy), folds the pointwise
self-connection into the center tap, and pre-arranges the input as a z-im2col
(3 z-shifted copies x 32 channels = 96 partitions) padded volume per batch.

Device side (per core, batch-parallel over 8 cores): a plain 3D conv as 9
accumulated matmuls (contraction K=96 = 3 z-taps x 32 ch) per 512-voxel output
chunk, PSUM accumulation, evacuate to SBUF, DMA out in [C_out, XYZ] layout.
Host transposes the output back to [B, X, Y, Z, C_out].
"""

import math

import numpy as np

# ---- problem constants (hardcoded; kernel.py must be self-contained) ----
MUL_IN, MUL_OUT = 8, 16
DIM_IN, DIM_OUT = 4 * MUL_IN, 4 * MUL_OUT  # 32, 64
DIAMETER = 3.0
NUM_RB = 4
BATCH, GRID = 8, 32
N_CORES = 8

XP = GRID + 2  # padded x planes: -1 .. 32
YP = GRID + 2  # padded y rows
PLANE = YP * GRID  # floats per (padded-y, z) plane = 34*32 = 1088
XFREE = XP * PLANE  # per-partition floats of the im2col tile = 36992
KPART = 3 * DIM_IN  # 96 partitions: z-shift blocks (dz=-1,0,+1) x 32 channels


# --------------------------------------------------------------------------
# host-side math: replicate the reference kernel build in fp32 numpy
# --------------------------------------------------------------------------
def _sus(x):
    # smooth unit step: exp(-1/x) for x>0 else 0
    safe = np.where(x > 0.0, x, 1.0).astype(np.float32)
    return np.where(x > 0.0, np.exp(np.float32(-1.0) / safe), np.float32(0.0))


def build_conv_kernel(w_lin0, w_lin1, w000, w011, w101, w110):
    """Returns K [3,3,3,DIM_IN,DIM_OUT] fp32 with the self-connection folded
    into the center tap."""
    f32 = np.float32
    r = DIAMETER / 2
    ax = np.arange(-math.floor(r), math.floor(r) + 1.0, dtype=f32)  # [-1,0,1]
    lattice = np.stack(np.meshgrid(ax, ax, ax, indexing="ij"), axis=-1).astype(f32)

    dist = np.linalg.norm(lattice, axis=-1).astype(f32)  # [3,3,3]
    values = np.linspace(0.0, DIAMETER / 2, NUM_RB + 2, dtype=f32)
    step = values[1] - values[0]
    diff = (dist[..., None] - values[1:-1]) / step  # [3,3,3,4]
    emb = (f32(1.14136) * np.exp(f32(2.0)) * _sus(diff + 1.0) * _sus(1.0 - diff)).astype(f32)

    norm = np.linalg.norm(lattice, axis=-1, keepdims=True).astype(f32)
    unit = lattice / np.where(norm == 0.0, f32(1.0), norm)
    sh1 = (np.sqrt(f32(3.0)) * unit).astype(f32)  # [3,3,3,3]

    n_lat = 27
    def rad(w):
        # emb [3,3,3,4] x w [4,8,1,16] -> [3,3,3,8,16]
        return (np.einsum("xyzk,kuvw->xyzuw", emb, w.astype(f32)) / f32(n_lat)).astype(f32)

    r000, r011, r101, r110 = rad(w000), rad(w011), rad(w101), rad(w110)

    inv_s3 = f32(1.0 / math.sqrt(3.0))
    alpha = f32(1.0 / math.sqrt(2.0 * MUL_IN))

    # in 0e x sh 0e -> out 0e   [3,3,3,8,16]
    k00 = (alpha * r000).astype(f32)
    # in 0e x sh 1o -> out 1o   [3,3,3,8,48]
    k01 = (alpha * inv_s3) * np.einsum("xyzuw,xyzm->xyzuwm", r011, sh1)
    k01 = k01.reshape(3, 3, 3, MUL_IN, 3 * MUL_OUT).astype(f32)
    # in 1o x sh 1o -> out 0e   [3,3,3,24,16]
    k10 = (alpha * inv_s3) * np.einsum("xyzuw,xyzi->xyzuiw", r110, sh1)
    k10 = k10.reshape(3, 3, 3, 3 * MUL_IN, MUL_OUT).astype(f32)
    # in 1o x sh 0e -> out 1o   [3,3,3,24,48]
    eye3 = np.eye(3, dtype=f32)
    k11 = (alpha * inv_s3) * np.einsum("xyzuw,im->xyzuiwm", r101, eye3)
    k11 = k11.reshape(3, 3, 3, 3 * MUL_IN, 3 * MUL_OUT).astype(f32)

    k = np.concatenate(
        [
            np.concatenate([k00, k01], axis=-1),
            np.concatenate([k10, k11], axis=-1),
        ],
        axis=-2,
    ).astype(f32)  # [3,3,3,32,64]

    # ---- self-connection folded into the center tap ----
    lin_norm = f32(1.0 / math.sqrt(MUL_IN))
    w_sc = np.zeros((DIM_IN, DIM_OUT), f32)
    w_sc[:MUL_IN, :MUL_OUT] = w_lin0.astype(f32) * lin_norm
    # rows 8+3u+i -> cols 16+3w+i
    for i in range(3):
        rows = MUL_IN + 3 * np.arange(MUL_IN) + i
        cols = MUL_OUT + 3 * np.arange(MUL_OUT) + i
        w_sc[np.ix_(rows, cols)] = w_lin1.astype(f32) * lin_norm
    k[1, 1, 1] += w_sc
    return k


def pack_weights(k):
    """[3,3,3,32,64] -> wk [96, 9*64]: tap group t=(kx,ky) holds rows
    32*kz+ci, cols co."""
    wk = np.zeros((KPART, 9 * DIM_OUT), np.float32)
    for kx in range(3):
        for ky in range(3):
            t = kx * 3 + ky
            blk = k[kx, ky]  # [3, 32, 64] (kz, ci, co)
            wk[:, t * DIM_OUT : (t + 1) * DIM_OUT] = blk.reshape(KPART, DIM_OUT)
    return wk


def build_im2col(xb):
    """xb [32,32,32,32] (X,Y,Z,C) -> xim [96, XFREE] fp32.

    Partition 32*j + c holds x[., ., z + (j-1), c] laid out as
    [xp 0..33][yp 0..33][z 0..31] with zero padding at xp/yp borders and
    z-shift edges."""
    xt = np.ascontiguousarray(xb.transpose(3, 0, 1, 2))  # [C, X, Y, Z]
    xim = np.zeros((KPART, XP, YP, GRID), np.float32)
    # j=0 (dz=-1): dest z 1..31 <- src z 0..30
    xim[0:32, 1:33, 1:33, 1:32] = xt[:, :, :, 0:31]
    # j=1 (dz=0)
    xim[32:64, 1:33, 1:33, :] = xt
    # j=2 (dz=+1): dest z 0..30 <- src z 1..31
    xim[64:96, 1:33, 1:33, 0:31] = xt[:, :, :, 1:32]
    return xim.reshape(KPART, XFREE)


# --------------------------------------------------------------------------
# device program
# --------------------------------------------------------------------------
_PROGRAM_CACHE = {}


def build_program(repeat_reg=False):
    """Builds the Bass program. Returns the Bacc object (compiled)."""
    import concourse.bass as bass  # noqa: F401  (kept for API parity)
    import concourse.mybir as mybir
    import concourse.tile as tile
    from concourse import bacc

    nc = bacc.Bacc(
        "TRN2",
        target_bir_lowering=False,
        debug=False,
        enable_asserts=True,
        num_devices=N_CORES,
    )
    xim_d = nc.dram_tensor("xim", [KPART, XFREE], mybir.dt.float32, kind="ExternalInput").ap()
    wk_d = nc.dram_tensor("wk", [KPART, 9 * DIM_OUT], mybir.dt.float32, kind="ExternalInput").ap()
    out_d = nc.dram_tensor("out", [DIM_OUT, GRID * GRID * GRID], mybir.dt.float32, kind="ExternalOutput").ap()

    with tile.TileContext(nc) as tc:
        emit_body(nc, tc, xim_d, wk_d, out_d)

    nc.compile()
    return nc


def emit_body(nc, tc, xim_d, wk_d, out_d):
    import concourse.mybir as mybir

    f32 = mybir.dt.float32
    with (
        tc.tile_pool(name="xim", bufs=1) as xim_pool,
        tc.tile_pool(name="wk", bufs=1) as wk_pool,
        tc.tile_pool(name="ob", bufs=4) as ob_pool,
        tc.tile_pool(name="ps", bufs=8, space="PSUM") as ps_pool,
    ):
        wk_t = wk_pool.tile([KPART, 9 * DIM_OUT], f32)
        nc.sync.dma_start(out=wk_t[:, :], in_=wk_d[:, :])

        xim_t = xim_pool.tile([KPART, XFREE], f32)
        # split the big input DMA into per-x-slab pieces so matmuls can start
        # before the whole volume lands
        N_IN_CHUNKS = 8
        rows = XFREE // N_IN_CHUNKS
        for ci in range(N_IN_CHUNKS):
            nc.sync.dma_start(
                out=xim_t[:, ci * rows : (ci + 1) * rows],
                in_=xim_d[:, ci * rows : (ci + 1) * rows],
            )

        # out chunk = half x-plane: 512 voxels
        for xi in range(GRID):
            for h in range(2):
                ps = ps_pool.tile([DIM_OUT, 512], f32)
                y0 = h * 16
                t = 0
                for kx in range(3):
                    plane_off = (xi + kx) * PLANE
                    for ky in range(3):
                        off = plane_off + (y0 + ky) * GRID
                        nc.tensor.matmul(
                            out=ps[:, :],
                            lhsT=wk_t[:, t * DIM_OUT : (t + 1) * DIM_OUT],
                            rhs=xim_t[:, off : off + 512],
                            start=(t == 0),
                            stop=(t == 8),
                        )
                        t += 1
                ob = ob_pool.tile([DIM_OUT, 512], f32)
                nc.vector.tensor_copy(ob[:, :], ps[:, :])
                nc.scalar.dma_start(
                    out=out_d[:, xi * 1024 + y0 * GRID : xi * 1024 + y0 * GRID + 512],
                    in_=ob[:, :],
                )


# --------------------------------------------------------------------------
# runner
# --------------------------------------------------------------------------
def _get_program():
    if "nc" not in _PROGRAM_CACHE:
        _PROGRAM_CACHE["nc"] = build_program()
    return _PROGRAM_CACHE["nc"]


def kernel(x, w_lin0, w_lin1, w000, w011, w101, w110):
    from concourse.bass_utils import run_bass_kernel_spmd

    x = np.asarray(x, np.float32)
    k = build_conv_kernel(
        np.asarray(w_lin0), np.asarray(w_lin1),
        np.asarray(w000), np.asarray(w011), np.asarray(w101), np.asarray(w110),
    )
    wk = pack_weights(k)

    in_maps = []
    for b in range(BATCH):
        in_maps.append({"xim": build_im2col(x[b]), "wk": wk})

    nc = _get_program()
    res = run_bass_kernel_spmd(nc, in_maps, list(range(N_CORES)))

    out = np.empty((BATCH, GRID, GRID, GRID, DIM_OUT), np.float32)
    for b in range(BATCH):
        ob = res.results[b]["out"]  # [64, 32768]
        out[b] = ob.reshape(DIM_OUT, GRID, GRID, GRID).transpose(1, 2, 3, 0)
    return out


# revision 3
# speedup vs baseline: 5.3333x; 5.3333x over previous
"""Trainium2 Bass kernel for the e3nn-style 3D convolution problem.

Host side: builds the tiny [3,3,3,32,64] conv kernel from the radial/spherical
weights (replicating the reference math in fp32 numpy), folds the pointwise
self-connection into the center tap, and pre-arranges the input as a z-im2col
(3 z-shifted copies x 32 channels = 96 partitions) padded volume per batch.

Device side (per core, batch-parallel over 8 cores): a plain 3D conv as 9
accumulated matmuls (contraction K=96 = 3 z-taps x 32 ch) per 512-voxel output
chunk, PSUM accumulation, evacuate to SBUF, DMA out in [C_out, XYZ] layout.
Host transposes the output back to [B, X, Y, Z, C_out].
"""

import math

import numpy as np

# ---- problem constants (hardcoded; kernel.py must be self-contained) ----
MUL_IN, MUL_OUT = 8, 16
DIM_IN, DIM_OUT = 4 * MUL_IN, 4 * MUL_OUT  # 32, 64
DIAMETER = 3.0
NUM_RB = 4
BATCH, GRID = 8, 32
N_CORES = 8

XP = GRID + 2  # padded x planes: -1 .. 32
YP = GRID + 2  # padded y rows
PLANE = YP * GRID  # floats per (padded-y, z) plane = 34*32 = 1088
XFREE = XP * PLANE  # per-partition floats of the im2col tile = 36992
KPART = 3 * DIM_IN  # 96 partitions: z-shift blocks (dz=-1,0,+1) x 32 channels


# --------------------------------------------------------------------------
# host-side math: replicate the reference kernel build in fp32 numpy
# --------------------------------------------------------------------------
def _sus(x):
    # smooth unit step: exp(-1/x) for x>0 else 0
    safe = np.where(x > 0.0, x, 1.0).astype(np.float32)
    return np.where(x > 0.0, np.exp(np.float32(-1.0) / safe), np.float32(0.0))


def build_conv_kernel(w_lin0, w_lin1, w000, w011, w101, w110):
    """Returns K [3,3,3,DIM_IN,DIM_OUT] fp32 with the self-connection folded
    into the center tap."""
    f32 = np.float32
    r = DIAMETER / 2
    ax = np.arange(-math.floor(r), math.floor(r) + 1.0, dtype=f32)  # [-1,0,1]
    lattice = np.stack(np.meshgrid(ax, ax, ax, indexing="ij"), axis=-1).astype(f32)

    dist = np.linalg.norm(lattice, axis=-1).astype(f32)  # [3,3,3]
    values = np.linspace(0.0, DIAMETER / 2, NUM_RB + 2, dtype=f32)
    step = values[1] - values[0]
    diff = (dist[..., None] - values[1:-1]) / step  # [3,3,3,4]
    emb = (f32(1.14136) * np.exp(f32(2.0)) * _sus(diff + 1.0) * _sus(1.0 - diff)).astype(f32)

    norm = np.linalg.norm(lattice, axis=-1, keepdims=True).astype(f32)
    unit = lattice / np.where(norm == 0.0, f32(1.0), norm)
    sh1 = (np.sqrt(f32(3.0)) * unit).astype(f32)  # [3,3,3,3]

    n_lat = 27
    def rad(w):
        # emb [3,3,3,4] x w [4,8,1,16] -> [3,3,3,8,16]
        return (np.einsum("xyzk,kuvw->xyzuw", emb, w.astype(f32)) / f32(n_lat)).astype(f32)

    r000, r011, r101, r110 = rad(w000), rad(w011), rad(w101), rad(w110)

    inv_s3 = f32(1.0 / math.sqrt(3.0))
    alpha = f32(1.0 / math.sqrt(2.0 * MUL_IN))

    # in 0e x sh 0e -> out 0e   [3,3,3,8,16]
    k00 = (alpha * r000).astype(f32)
    # in 0e x sh 1o -> out 1o   [3,3,3,8,48]
    k01 = (alpha * inv_s3) * np.einsum("xyzuw,xyzm->xyzuwm", r011, sh1)
    k01 = k01.reshape(3, 3, 3, MUL_IN, 3 * MUL_OUT).astype(f32)
    # in 1o x sh 1o -> out 0e   [3,3,3,24,16]
    k10 = (alpha * inv_s3) * np.einsum("xyzuw,xyzi->xyzuiw", r110, sh1)
    k10 = k10.reshape(3, 3, 3, 3 * MUL_IN, MUL_OUT).astype(f32)
    # in 1o x sh 0e -> out 1o   [3,3,3,24,48]
    eye3 = np.eye(3, dtype=f32)
    k11 = (alpha * inv_s3) * np.einsum("xyzuw,im->xyzuiwm", r101, eye3)
    k11 = k11.reshape(3, 3, 3, 3 * MUL_IN, 3 * MUL_OUT).astype(f32)

    k = np.concatenate(
        [
            np.concatenate([k00, k01], axis=-1),
            np.concatenate([k10, k11], axis=-1),
        ],
        axis=-2,
    ).astype(f32)  # [3,3,3,32,64]

    # ---- self-connection folded into the center tap ----
    lin_norm = f32(1.0 / math.sqrt(MUL_IN))
    w_sc = np.zeros((DIM_IN, DIM_OUT), f32)
    w_sc[:MUL_IN, :MUL_OUT] = w_lin0.astype(f32) * lin_norm
    # rows 8+3u+i -> cols 16+3w+i
    for i in range(3):
        rows = MUL_IN + 3 * np.arange(MUL_IN) + i
        cols = MUL_OUT + 3 * np.arange(MUL_OUT) + i
        w_sc[np.ix_(rows, cols)] = w_lin1.astype(f32) * lin_norm
    k[1, 1, 1] += w_sc
    return k


def pack_weights(k):
    """[3,3,3,32,64] -> wk [96, 9*64]: tap group t=(kx,ky) holds rows
    32*kz+ci, cols co."""
    wk = np.zeros((KPART, 9 * DIM_OUT), np.float32)
    for kx in range(3):
        for ky in range(3):
            t = kx * 3 + ky
            blk = k[kx, ky]  # [3, 32, 64] (kz, ci, co)
            wk[:, t * DIM_OUT : (t + 1) * DIM_OUT] = blk.reshape(KPART, DIM_OUT)
    return wk


def build_im2col(xb):
    """xb [32,32,32,32] (X,Y,Z,C) -> xim [96, XFREE] fp32.

    Partition 32*j + c holds x[., ., z + (j-1), c] laid out as
    [xp 0..33][yp 0..33][z 0..31] with zero padding at xp/yp borders and
    z-shift edges."""
    xt = np.ascontiguousarray(xb.transpose(3, 0, 1, 2))  # [C, X, Y, Z]
    xim = np.zeros((KPART, XP, YP, GRID), np.float32)
    # j=0 (dz=-1): dest z 1..31 <- src z 0..30
    xim[0:32, 1:33, 1:33, 1:32] = xt[:, :, :, 0:31]
    # j=1 (dz=0)
    xim[32:64, 1:33, 1:33, :] = xt
    # j=2 (dz=+1): dest z 0..30 <- src z 1..31
    xim[64:96, 1:33, 1:33, 0:31] = xt[:, :, :, 1:32]
    return xim.reshape(KPART, XFREE)


# --------------------------------------------------------------------------
# device program
# --------------------------------------------------------------------------
_PROGRAM_CACHE = {}

# matmul operand dtype: "fp32r" = full PE rate with ~1.4e-4 relative error,
# "fp32" = exact but 1/4 PE rate.
MM_DTYPE = "fp32r"


def _mm_dt(mybir):
    return mybir.dt.float32r if MM_DTYPE == "fp32r" else mybir.dt.float32


def build_program():
    """Builds the Bass program. Returns the Bacc object (compiled)."""
    import concourse.bass as bass  # noqa: F401  (kept for API parity)
    import concourse.mybir as mybir
    import concourse.tile as tile
    from concourse import bacc

    nc = bacc.Bacc(
        "TRN2",
        target_bir_lowering=False,
        debug=False,
        enable_asserts=True,
        num_devices=N_CORES,
    )
    mdt = _mm_dt(mybir)
    xim_d = nc.dram_tensor("xim", [KPART, XFREE], mdt, kind="ExternalInput").ap()
    wk_d = nc.dram_tensor("wk", [KPART, 9 * DIM_OUT], mdt, kind="ExternalInput").ap()
    out_d = nc.dram_tensor("out", [DIM_OUT, GRID * GRID * GRID], mybir.dt.float32, kind="ExternalOutput").ap()

    with tile.TileContext(nc) as tc:
        emit_body(nc, tc, xim_d, wk_d, out_d)

    nc.compile()
    return nc


def emit_body(nc, tc, xim_d, wk_d, out_d):
    import concourse.mybir as mybir

    f32 = mybir.dt.float32
    mdt = _mm_dt(mybir)
    with (
        tc.tile_pool(name="xim", bufs=1) as xim_pool,
        tc.tile_pool(name="wk", bufs=1) as wk_pool,
        tc.tile_pool(name="ob", bufs=4) as ob_pool,
        tc.tile_pool(name="ps", bufs=8, space="PSUM") as ps_pool,
    ):
        wk_t = wk_pool.tile([KPART, 9 * DIM_OUT], mdt)
        nc.sync.dma_start(out=wk_t[:, :], in_=wk_d[:, :])

        xim_t = xim_pool.tile([KPART, XFREE], mdt)
        # split the big input DMA into per-x-slab pieces so matmuls can start
        # before the whole volume lands
        N_IN_CHUNKS = 8
        rows = XFREE // N_IN_CHUNKS
        for ci in range(N_IN_CHUNKS):
            nc.sync.dma_start(
                out=xim_t[:, ci * rows : (ci + 1) * rows],
                in_=xim_d[:, ci * rows : (ci + 1) * rows],
            )

        # out chunk = half x-plane: 512 voxels
        for xi in range(GRID):
            for h in range(2):
                ps = ps_pool.tile([DIM_OUT, 512], f32)
                y0 = h * 16
                t = 0
                for kx in range(3):
                    plane_off = (xi + kx) * PLANE
                    for ky in range(3):
                        off = plane_off + (y0 + ky) * GRID
                        nc.tensor.matmul(
                            out=ps[:, :],
                            lhsT=wk_t[:, t * DIM_OUT : (t + 1) * DIM_OUT],
                            rhs=xim_t[:, off : off + 512],
                            start=(t == 0),
                            stop=(t == 8),
                        )
                        t += 1
                ob = ob_pool.tile([DIM_OUT, 512], f32)
                nc.vector.tensor_copy(ob[:, :], ps[:, :])
                nc.scalar.dma_start(
                    out=out_d[:, xi * 1024 + y0 * GRID : xi * 1024 + y0 * GRID + 512],
                    in_=ob[:, :],
                )


# --------------------------------------------------------------------------
# runner
# --------------------------------------------------------------------------
def _get_program():
    if "nc" not in _PROGRAM_CACHE:
        _PROGRAM_CACHE["nc"] = build_program()
    return _PROGRAM_CACHE["nc"]


def kernel(x, w_lin0, w_lin1, w000, w011, w101, w110):
    from concourse.bass_utils import run_bass_kernel_spmd

    x = np.asarray(x, np.float32)
    k = build_conv_kernel(
        np.asarray(w_lin0), np.asarray(w_lin1),
        np.asarray(w000), np.asarray(w011), np.asarray(w101), np.asarray(w110),
    )
    wk = pack_weights(k)

    in_maps = []
    for b in range(BATCH):
        in_maps.append({"xim": build_im2col(x[b]), "wk": wk})

    nc = _get_program()
    res = run_bass_kernel_spmd(nc, in_maps, list(range(N_CORES)))

    out = np.empty((BATCH, GRID, GRID, GRID, DIM_OUT), np.float32)
    for b in range(BATCH):
        ob = res.results[b]["out"]  # [64, 32768]
        out[b] = ob.reshape(DIM_OUT, GRID, GRID, GRID).transpose(1, 2, 3, 0)
    return out


# revision 9
# speedup vs baseline: 6.0223x; 1.1292x over previous
"""Trainium2 Bass kernel for the e3nn-style 3D convolution problem.

Host side: builds the tiny [3,3,3,32,64] conv kernel from the radial/spherical
weights (replicating the reference math in fp32 numpy), folds the pointwise
self-connection into the center tap, and pre-arranges the input as a z-im2col
(3 z-shifted copies x 32 channels = 96 partitions) padded volume per batch.

Device side (per core, batch-parallel over 8 cores): a plain 3D conv as 9
accumulated matmuls (contraction K=96 = 3 z-taps x 32 ch) per 512-voxel output
chunk, PSUM accumulation, evacuate to SBUF, DMA out in [C_out, XYZ] layout.
Host transposes the output back to [B, X, Y, Z, C_out].
"""

import math

import numpy as np

# ---- problem constants (hardcoded; kernel.py must be self-contained) ----
MUL_IN, MUL_OUT = 8, 16
DIM_IN, DIM_OUT = 4 * MUL_IN, 4 * MUL_OUT  # 32, 64
DIAMETER = 3.0
NUM_RB = 4
BATCH, GRID = 8, 32
N_CORES = 8

XP = GRID + 2  # padded x planes: -1 .. 32
YP = GRID + 2  # padded y rows
PLANE = YP * GRID  # floats per (padded-y, z) plane = 34*32 = 1088
XFREE = XP * PLANE  # per-partition floats of the im2col tile = 36992
KPART = 3 * DIM_IN  # 96 partitions: z-shift blocks (dz=-1,0,+1) x 32 channels


# --------------------------------------------------------------------------
# host-side math: replicate the reference kernel build in fp32 numpy
# --------------------------------------------------------------------------
def _sus(x):
    # smooth unit step: exp(-1/x) for x>0 else 0
    safe = np.where(x > 0.0, x, 1.0).astype(np.float32)
    return np.where(x > 0.0, np.exp(np.float32(-1.0) / safe), np.float32(0.0))


def build_conv_kernel(w_lin0, w_lin1, w000, w011, w101, w110):
    """Returns K [3,3,3,DIM_IN,DIM_OUT] fp32 with the self-connection folded
    into the center tap."""
    f32 = np.float32
    r = DIAMETER / 2
    ax = np.arange(-math.floor(r), math.floor(r) + 1.0, dtype=f32)  # [-1,0,1]
    lattice = np.stack(np.meshgrid(ax, ax, ax, indexing="ij"), axis=-1).astype(f32)

    dist = np.linalg.norm(lattice, axis=-1).astype(f32)  # [3,3,3]
    values = np.linspace(0.0, DIAMETER / 2, NUM_RB + 2, dtype=f32)
    step = values[1] - values[0]
    diff = (dist[..., None] - values[1:-1]) / step  # [3,3,3,4]
    emb = (f32(1.14136) * np.exp(f32(2.0)) * _sus(diff + 1.0) * _sus(1.0 - diff)).astype(f32)

    norm = np.linalg.norm(lattice, axis=-1, keepdims=True).astype(f32)
    unit = lattice / np.where(norm == 0.0, f32(1.0), norm)
    sh1 = (np.sqrt(f32(3.0)) * unit).astype(f32)  # [3,3,3,3]

    n_lat = 27
    def rad(w):
        # emb [3,3,3,4] x w [4,8,1,16] -> [3,3,3,8,16]
        return (np.einsum("xyzk,kuvw->xyzuw", emb, w.astype(f32)) / f32(n_lat)).astype(f32)

    r000, r011, r101, r110 = rad(w000), rad(w011), rad(w101), rad(w110)

    inv_s3 = f32(1.0 / math.sqrt(3.0))
    alpha = f32(1.0 / math.sqrt(2.0 * MUL_IN))

    # in 0e x sh 0e -> out 0e   [3,3,3,8,16]
    k00 = (alpha * r000).astype(f32)
    # in 0e x sh 1o -> out 1o   [3,3,3,8,48]
    k01 = (alpha * inv_s3) * np.einsum("xyzuw,xyzm->xyzuwm", r011, sh1)
    k01 = k01.reshape(3, 3, 3, MUL_IN, 3 * MUL_OUT).astype(f32)
    # in 1o x sh 1o -> out 0e   [3,3,3,24,16]
    k10 = (alpha * inv_s3) * np.einsum("xyzuw,xyzi->xyzuiw", r110, sh1)
    k10 = k10.reshape(3, 3, 3, 3 * MUL_IN, MUL_OUT).astype(f32)
    # in 1o x sh 0e -> out 1o   [3,3,3,24,48]
    eye3 = np.eye(3, dtype=f32)
    k11 = (alpha * inv_s3) * np.einsum("xyzuw,im->xyzuiwm", r101, eye3)
    k11 = k11.reshape(3, 3, 3, 3 * MUL_IN, 3 * MUL_OUT).astype(f32)

    k = np.concatenate(
        [
            np.concatenate([k00, k01], axis=-1),
            np.concatenate([k10, k11], axis=-1),
        ],
        axis=-2,
    ).astype(f32)  # [3,3,3,32,64]

    # ---- self-connection folded into the center tap ----
    lin_norm = f32(1.0 / math.sqrt(MUL_IN))
    w_sc = np.zeros((DIM_IN, DIM_OUT), f32)
    w_sc[:MUL_IN, :MUL_OUT] = w_lin0.astype(f32) * lin_norm
    # rows 8+3u+i -> cols 16+3w+i
    for i in range(3):
        rows = MUL_IN + 3 * np.arange(MUL_IN) + i
        cols = MUL_OUT + 3 * np.arange(MUL_OUT) + i
        w_sc[np.ix_(rows, cols)] = w_lin1.astype(f32) * lin_norm
    k[1, 1, 1] += w_sc
    return k


def pack_weights(k):
    """[3,3,3,32,64] -> wk [96, 576] in the dx-paired layout:

    cols [128*ky, 128*ky+64)    = tap (kx=0, ky)  rows 32*kz+ci
    cols [128*ky+64, 128*ky+128) = tap (kx=2, ky)
    cols [384+64*ky, 384+64*ky+64) = tap (kx=1, ky)  ("singles")
    """
    wk = np.zeros((KPART, 9 * DIM_OUT), np.float32)
    for ky in range(3):
        wk[:, 128 * ky : 128 * ky + 64] = k[0, ky].reshape(KPART, DIM_OUT)
        wk[:, 128 * ky + 64 : 128 * ky + 128] = k[2, ky].reshape(KPART, DIM_OUT)
        wk[:, 384 + 64 * ky : 384 + 64 * (ky + 1)] = k[1, ky].reshape(KPART, DIM_OUT)
    return wk


def build_im2col(xb):
    """xb [32,32,32,32] (X,Y,Z,C) -> xim [96, XFREE] fp32.

    Partition 32*j + c holds x[., ., z + (j-1), c] laid out as
    [xp 0..33][yp 0..33][z 0..31] with zero padding at xp/yp borders and
    z-shift edges."""
    xt = np.ascontiguousarray(xb.transpose(3, 0, 1, 2))  # [C, X, Y, Z]
    xim = np.zeros((KPART, XP, YP, GRID), np.float32)
    # j=0 (dz=-1): dest z 1..31 <- src z 0..30
    xim[0:32, 1:33, 1:33, 1:32] = xt[:, :, :, 0:31]
    # j=1 (dz=0)
    xim[32:64, 1:33, 1:33, :] = xt
    # j=2 (dz=+1): dest z 0..30 <- src z 1..31
    xim[64:96, 1:33, 1:33, 0:31] = xt[:, :, :, 1:32]
    return xim.reshape(KPART, XFREE)


# --------------------------------------------------------------------------
# device program
# --------------------------------------------------------------------------
_PROGRAM_CACHE = {}

# matmul operand dtype: "fp32r" = full PE rate with ~1.4e-4 relative error,
# "fp32" = exact but 1/4 PE rate.
MM_DTYPE = "fp32r"


def _mm_dt(mybir):
    return mybir.dt.float32r if MM_DTYPE == "fp32r" else mybir.dt.float32


def build_program():
    """Builds the Bass program. Returns the Bacc object (compiled)."""
    import concourse.bass as bass  # noqa: F401  (kept for API parity)
    import concourse.mybir as mybir
    import concourse.tile as tile
    from concourse import bacc

    nc = bacc.Bacc(
        "TRN2",
        target_bir_lowering=False,
        debug=False,
        enable_asserts=True,
        num_devices=N_CORES,
    )
    mdt = _mm_dt(mybir)
    xim_d = nc.dram_tensor("xim", [KPART, XFREE], mdt, kind="ExternalInput").ap()
    wk_d = nc.dram_tensor("wk", [KPART, 9 * DIM_OUT], mdt, kind="ExternalInput").ap()
    out_d = nc.dram_tensor("out", [DIM_OUT, GRID * GRID * GRID], mybir.dt.float32, kind="ExternalOutput").ap()

    with tile.TileContext(nc) as tc:
        emit_body(nc, tc, xim_d, wk_d, out_d)

    nc.compile()
    return nc


def emit_body(nc, tc, xim_d, wk_d, out_d):
    """dx-paired scheme: stationary M=128 holds [W(dx=-1) | W(dx=+1)] per ky.

    For out-plane group xi (streaming base plane xp=xi, i.e. x[xi-1]):
      rows 0-63   accumulate taps (kx=0, ky) for out plane xi
      rows 64-127 accumulate taps (kx=2, ky) for out plane xi-2
    plus 3 "single" matmuls (kx=1) into rows 0-63 (base plane xp=xi+1).
    Evacuation of out plane xi adds bank[xi][0:64] + bank[xi+2][64:128].
    """
    import concourse.mybir as mybir

    f32 = mybir.dt.float32
    mdt = _mm_dt(mybir)

    N_SLABS = 4
    SLAB_OUT = GRID // N_SLABS  # 8 out planes per slab
    SLAB_PLANES = SLAB_OUT + 2  # 10 padded planes resident per slab tile

    with (
        tc.tile_pool(name="xs", bufs=N_SLABS) as xs_pool,
        tc.tile_pool(name="wk", bufs=1) as wk_pool,
        tc.tile_pool(name="ob", bufs=4) as ob_pool,
        tc.tile_pool(name="ps", bufs=8, space="PSUM") as ps_pool,
    ):
        wk_t = wk_pool.tile([KPART, 9 * DIM_OUT], mdt)
        nc.sync.dma_start(out=wk_t[:, :], in_=wk_d[:, :])

        # weight column layout in wk_d (from pack_weights): tap t = kx*3+ky
        # pair stationary for ky: cols of tap (0,ky) | tap (2,ky)  -> need
        # them adjacent. We instead issue the pair matmul with a single AP
        # over a repacked weight tile: host packs wkp with layout
        #   [pair ky=0 (128)] [pair ky=1 (128)] [pair ky=2 (128)]
        #   [single ky=0 (64)] [single ky=1 (64)] [single ky=2 (64)]
        # (see pack_weights_paired). wk_d already holds that layout.

        slabs = []
        for s in range(N_SLABS):
            xs = xs_pool.tile([KPART, SLAB_PLANES * PLANE], mdt)
            base = (s * SLAB_OUT) * PLANE
            nc.sync.dma_start(
                out=xs[:, :], in_=xim_d[:, base : base + SLAB_PLANES * PLANE]
            )
            slabs.append(xs)

        def rhs_slice(xp, y):
            """contiguous [96, 512] covering 16 y-rows from padded plane xp,
            y-row start y (0..33)."""
            s = min(max((xp - 0) // SLAB_OUT, 0), N_SLABS - 1)
            # slab s covers padded planes [s*8, s*8+10)
            if xp >= s * SLAB_OUT + SLAB_PLANES:
                s += 1
            if xp < s * SLAB_OUT:
                s -= 1
            loc = xp - s * SLAB_OUT
            off = loc * PLANE + y * GRID
            return slabs[s][:, off : off + 512]

        banks = {}  # (xi, h) -> psum tile
        obs = []

        def evac(xi, h):
            y0 = h * 16
            ob = ob_pool.tile([DIM_OUT, 512], f32)
            pa = banks[(xi, h)]
            nc.vector.tensor_copy(ob[:, :], pa[0:DIM_OUT, :])
            if xi < GRID - 1:
                pb = banks[(xi + 2, h)]
                nc.vector.tensor_add(ob[:, :], ob[:, :], pb[DIM_OUT : 2 * DIM_OUT, :])
            nc.scalar.dma_start(
                out=out_d[:, xi * 1024 + y0 * GRID : xi * 1024 + y0 * GRID + 512],
                in_=ob[:, :],
            )
            del banks[(xi, h)]

        # groups xi = 0..32; group 32 runs pairs only (feeds out plane 30)
        for xb in range(GRID // 2 + 1):  # blocks of up to 2 plane-groups
            gxs = [g for g in (2 * xb, 2 * xb + 1) if g <= GRID]
            for g in gxs:
                for h in (0, 1):
                    banks[(g, h)] = ps_pool.tile(
                        [2 * DIM_OUT, 512], f32, name=f"bank_{g}_{h}", tag="bank"
                    )
            # weight-major inner order: load each stationary once per block
            for w in range(6):
                for xi in gxs:
                    if xi == GRID and w >= 3:
                        continue  # group 32: pairs only
                    for h in (0, 1):
                        y0 = h * 16
                        ps = banks[(xi, h)]
                        if w < 3:  # pair ky=w, base plane xp=xi
                            ky = w
                            nc.tensor.matmul(
                                out=ps[:, :],
                                lhsT=wk_t[:, 128 * ky : 128 * (ky + 1)],
                                rhs=rhs_slice(xi, y0 + ky),
                                start=(w == 0),
                                stop=(w == 5) or (xi == GRID and w == 2),
                            )
                        else:  # single ky=w-3, base plane xp=xi+1
                            ky = w - 3
                            nc.tensor.matmul(
                                out=ps[0:DIM_OUT, :],
                                lhsT=wk_t[:, 384 + 64 * ky : 384 + 64 * (ky + 1)],
                                rhs=rhs_slice(xi + 1, y0 + ky),
                                start=False,
                                stop=(w == 5),
                            )
            # evacuate planes whose dependencies are now complete:
            # after block xb (groups 2xb, 2xb+1), planes 2xb-2 and 2xb-1 are ready
            for g in gxs:
                xr = g - 2
                if 0 <= xr < GRID:
                    for h in (0, 1):
                        evac(xr, h)
        # tail: plane 31 depends only on its own bank; group-32 banks held
        # rows 64-127 for plane 30 and are dropped now
        for h in (0, 1):
            evac(GRID - 1, h)
            del banks[(GRID, h)]
        assert not banks, f"unevacuated banks: {list(banks)}"


# --------------------------------------------------------------------------
# runner
# --------------------------------------------------------------------------
def _get_program():
    if "nc" not in _PROGRAM_CACHE:
        _PROGRAM_CACHE["nc"] = build_program()
    return _PROGRAM_CACHE["nc"]


def kernel(x, w_lin0, w_lin1, w000, w011, w101, w110):
    from concourse.bass_utils import run_bass_kernel_spmd

    x = np.asarray(x, np.float32)
    k = build_conv_kernel(
        np.asarray(w_lin0), np.asarray(w_lin1),
        np.asarray(w000), np.asarray(w011), np.asarray(w101), np.asarray(w110),
    )
    wk = pack_weights(k)

    in_maps = []
    for b in range(BATCH):
        in_maps.append({"xim": build_im2col(x[b]), "wk": wk})

    nc = _get_program()
    res = run_bass_kernel_spmd(nc, in_maps, list(range(N_CORES)))

    out = np.empty((BATCH, GRID, GRID, GRID, DIM_OUT), np.float32)
    for b in range(BATCH):
        ob = res.results[b]["out"]  # [64, 32768]
        out[b] = ob.reshape(DIM_OUT, GRID, GRID, GRID).transpose(1, 2, 3, 0)
    return out


# revision 24
# speedup vs baseline: 6.2081x; 1.0308x over previous
"""Trainium2 Bass kernel for the e3nn-style 3D convolution problem.

Host side: builds the tiny [3,3,3,32,64] conv kernel from the radial/spherical
weights (replicating the reference math in fp32 numpy), folds the pointwise
self-connection into the center tap, and pre-arranges the input as a z-im2col
(3 z-shifted copies x 32 channels = 96 partitions) padded volume per batch.

Device side (per core, batch-parallel over 8 cores): a plain 3D conv as 9
accumulated matmuls (contraction K=96 = 3 z-taps x 32 ch) per 512-voxel output
chunk, PSUM accumulation, evacuate to SBUF, DMA out in [C_out, XYZ] layout.
Host transposes the output back to [B, X, Y, Z, C_out].
"""

import math

import numpy as np

# ---- problem constants (hardcoded; kernel.py must be self-contained) ----
MUL_IN, MUL_OUT = 8, 16
DIM_IN, DIM_OUT = 4 * MUL_IN, 4 * MUL_OUT  # 32, 64
DIAMETER = 3.0
NUM_RB = 4
BATCH, GRID = 8, 32
N_CORES = 8

XP = GRID + 2  # padded x planes: -1 .. 32
YP = GRID + 2  # padded y rows
PLANE = YP * GRID  # floats per (padded-y, z) plane = 34*32 = 1088
XFREE = XP * PLANE  # per-partition floats of the im2col tile = 36992
KPART = 3 * DIM_IN  # 96 partitions: z-shift blocks (dz=-1,0,+1) x 32 channels


# --------------------------------------------------------------------------
# host-side math: replicate the reference kernel build in fp32 numpy
# --------------------------------------------------------------------------
def _sus(x):
    # smooth unit step: exp(-1/x) for x>0 else 0
    safe = np.where(x > 0.0, x, 1.0).astype(np.float32)
    return np.where(x > 0.0, np.exp(np.float32(-1.0) / safe), np.float32(0.0))


def build_conv_kernel(w_lin0, w_lin1, w000, w011, w101, w110):
    """Returns K [3,3,3,DIM_IN,DIM_OUT] fp32 with the self-connection folded
    into the center tap."""
    f32 = np.float32
    r = DIAMETER / 2
    ax = np.arange(-math.floor(r), math.floor(r) + 1.0, dtype=f32)  # [-1,0,1]
    lattice = np.stack(np.meshgrid(ax, ax, ax, indexing="ij"), axis=-1).astype(f32)

    dist = np.linalg.norm(lattice, axis=-1).astype(f32)  # [3,3,3]
    values = np.linspace(0.0, DIAMETER / 2, NUM_RB + 2, dtype=f32)
    step = values[1] - values[0]
    diff = (dist[..., None] - values[1:-1]) / step  # [3,3,3,4]
    emb = (f32(1.14136) * np.exp(f32(2.0)) * _sus(diff + 1.0) * _sus(1.0 - diff)).astype(f32)

    norm = np.linalg.norm(lattice, axis=-1, keepdims=True).astype(f32)
    unit = lattice / np.where(norm == 0.0, f32(1.0), norm)
    sh1 = (np.sqrt(f32(3.0)) * unit).astype(f32)  # [3,3,3,3]

    n_lat = 27
    def rad(w):
        # emb [3,3,3,4] x w [4,8,1,16] -> [3,3,3,8,16]
        return (np.einsum("xyzk,kuvw->xyzuw", emb, w.astype(f32)) / f32(n_lat)).astype(f32)

    r000, r011, r101, r110 = rad(w000), rad(w011), rad(w101), rad(w110)

    inv_s3 = f32(1.0 / math.sqrt(3.0))
    alpha = f32(1.0 / math.sqrt(2.0 * MUL_IN))

    # in 0e x sh 0e -> out 0e   [3,3,3,8,16]
    k00 = (alpha * r000).astype(f32)
    # in 0e x sh 1o -> out 1o   [3,3,3,8,48]
    k01 = (alpha * inv_s3) * np.einsum("xyzuw,xyzm->xyzuwm", r011, sh1)
    k01 = k01.reshape(3, 3, 3, MUL_IN, 3 * MUL_OUT).astype(f32)
    # in 1o x sh 1o -> out 0e   [3,3,3,24,16]
    k10 = (alpha * inv_s3) * np.einsum("xyzuw,xyzi->xyzuiw", r110, sh1)
    k10 = k10.reshape(3, 3, 3, 3 * MUL_IN, MUL_OUT).astype(f32)
    # in 1o x sh 0e -> out 1o   [3,3,3,24,48]
    eye3 = np.eye(3, dtype=f32)
    k11 = (alpha * inv_s3) * np.einsum("xyzuw,im->xyzuiwm", r101, eye3)
    k11 = k11.reshape(3, 3, 3, 3 * MUL_IN, 3 * MUL_OUT).astype(f32)

    k = np.concatenate(
        [
            np.concatenate([k00, k01], axis=-1),
            np.concatenate([k10, k11], axis=-1),
        ],
        axis=-2,
    ).astype(f32)  # [3,3,3,32,64]

    # ---- self-connection folded into the center tap ----
    lin_norm = f32(1.0 / math.sqrt(MUL_IN))
    w_sc = np.zeros((DIM_IN, DIM_OUT), f32)
    w_sc[:MUL_IN, :MUL_OUT] = w_lin0.astype(f32) * lin_norm
    # rows 8+3u+i -> cols 16+3w+i
    for i in range(3):
        rows = MUL_IN + 3 * np.arange(MUL_IN) + i
        cols = MUL_OUT + 3 * np.arange(MUL_OUT) + i
        w_sc[np.ix_(rows, cols)] = w_lin1.astype(f32) * lin_norm
    k[1, 1, 1] += w_sc
    return k


def pack_weights(k):
    """[3,3,3,32,64] -> wk [96, 576] in the dx-paired layout:

    cols [128*ky, 128*ky+64)    = tap (kx=0, ky)  rows 32*kz+ci
    cols [128*ky+64, 128*ky+128) = tap (kx=2, ky)
    cols [384+64*ky, 384+64*ky+64) = tap (kx=1, ky)  ("singles")
    """
    wk = np.zeros((KPART, 9 * DIM_OUT), np.float32)
    for ky in range(3):
        wk[:, 128 * ky : 128 * ky + 64] = k[0, ky].reshape(KPART, DIM_OUT)
        wk[:, 128 * ky + 64 : 128 * ky + 128] = k[2, ky].reshape(KPART, DIM_OUT)
        wk[:, 384 + 64 * ky : 384 + 64 * (ky + 1)] = k[1, ky].reshape(KPART, DIM_OUT)
    return wk


def build_im2col(xb):
    """xb [32,32,32,32] (X,Y,Z,C) -> xim [96, XFREE] fp32.

    Partition 32*j + c holds x[., ., z + (j-1), c] laid out as
    [xp 0..33][yp 0..33][z 0..31] with zero padding at xp/yp borders and
    z-shift edges."""
    xt = np.ascontiguousarray(xb.transpose(3, 0, 1, 2))  # [C, X, Y, Z]
    xim = np.zeros((KPART, XP, YP, GRID), np.float32)
    # j=0 (dz=-1): dest z 1..31 <- src z 0..30
    xim[0:32, 1:33, 1:33, 1:32] = xt[:, :, :, 0:31]
    # j=1 (dz=0)
    xim[32:64, 1:33, 1:33, :] = xt
    # j=2 (dz=+1): dest z 0..30 <- src z 1..31
    xim[64:96, 1:33, 1:33, 0:31] = xt[:, :, :, 1:32]
    return xim.reshape(KPART, XFREE)


def build_xpad(xb):
    """Replication-mode host input: just the dz=0 block, [32, XFREE]."""
    xt = np.ascontiguousarray(xb.transpose(3, 0, 1, 2))
    xp = np.zeros((DIM_IN, XP, YP, GRID), np.float32)
    xp[:, 1:33, 1:33, :] = xt
    return xp.reshape(DIM_IN, XFREE)


# --------------------------------------------------------------------------
# device program
# --------------------------------------------------------------------------
_PROGRAM_CACHE = {}

# matmul operand dtype: "fp32r" = full PE rate with ~1.4e-4 relative error,
# "fp32" = exact but 1/4 PE rate.
MM_DTYPE = "fp32r"


def _mm_dt(mybir):
    return mybir.dt.float32r if MM_DTYPE == "fp32r" else mybir.dt.float32


def build_program():
    """Builds the Bass program. Returns the Bacc object (compiled)."""
    import concourse.bass as bass  # noqa: F401  (kept for API parity)
    import concourse.mybir as mybir
    import concourse.tile as tile
    from concourse import bacc

    nc = bacc.Bacc(
        "TRN2",
        target_bir_lowering=False,
        debug=False,
        enable_asserts=True,
        num_devices=N_CORES,
    )
    mdt = _mm_dt(mybir)
    xim_d = nc.dram_tensor("xim", [KPART, XFREE], mdt, kind="ExternalInput").ap()
    wk_d = nc.dram_tensor("wk", [KPART, 9 * DIM_OUT], mdt, kind="ExternalInput").ap()
    out_d = nc.dram_tensor("out", [DIM_OUT, GRID * GRID * GRID], mybir.dt.float32, kind="ExternalOutput").ap()

    with tile.TileContext(nc) as tc:
        emit_body(nc, tc, xim_d, wk_d, out_d)

    nc.compile()
    return nc


def emit_body(nc, tc, xim_d, wk_d, out_d, mode="full"):
    """dx-paired scheme: stationary M=128 holds [W(dx=-1) | W(dx=+1)] per ky.

    For out-plane group xi (streaming base plane xp=xi, i.e. x[xi-1]):
      rows 0-63   accumulate taps (kx=0, ky) for out plane xi
      rows 64-127 accumulate taps (kx=2, ky) for out plane xi-2
    plus 3 "single" matmuls (kx=1) into rows 0-63 (base plane xp=xi+1).
    Evacuation of out plane xi adds bank[xi][0:64] + bank[xi+2][64:128].
    """
    import concourse.mybir as mybir

    f32 = mybir.dt.float32
    mdt = _mm_dt(mybir)

    do_in = mode not in ("noin", "mmpure")
    do_mm = mode not in ("dma",)
    do_evac = mode in ("full", "noin", "dma")
    one_weight = mode == "mm1w"

    N_SLABS = globals().get("N_SLABS_OVR", 4)
    SLAB_OUT = GRID // N_SLABS  # out planes per slab
    SLAB_PLANES = SLAB_OUT + 2  # padded planes resident per slab tile
    ob_bufs = globals().get("OB_BUFS_OVR", 8)
    out_eng = getattr(nc, globals().get("OUT_ENGINE", "sync"))
    in_eng = getattr(nc, globals().get("IN_ENGINE", "sync"))

    with (
        tc.tile_pool(name="xs", bufs=N_SLABS) as xs_pool,
        tc.tile_pool(name="wk", bufs=1) as wk_pool,
        tc.tile_pool(name="ob", bufs=ob_bufs) as ob_pool,
        tc.tile_pool(name="ps", bufs=8, space="PSUM") as ps_pool,
    ):
        wk_t = wk_pool.tile([KPART, 9 * DIM_OUT], mdt)
        nc.sync.dma_start(out=wk_t[:, :], in_=wk_d[:, :])

        if mode == "dma":
            # input DMAs + output DMAs of a constant tile, no compute
            slabs = []
            for s in range(N_SLABS):
                xs = xs_pool.tile([KPART, SLAB_PLANES * PLANE], mdt)
                base = (s * SLAB_OUT) * PLANE
                nc.sync.dma_start(out=xs[:, :], in_=xim_d[:, base : base + SLAB_PLANES * PLANE])
                slabs.append(xs)
            ob0 = ob_pool.tile([DIM_OUT, 512], f32)
            nc.vector.memset(ob0[:, :], 0.0)
            for xi in range(GRID):
                for h in (0, 1):
                    y0 = h * 16
                    out_eng.dma_start(
                        out=out_d[:, xi * 1024 + y0 * GRID : xi * 1024 + y0 * GRID + 512],
                        in_=ob0[:, :],
                    )
            return

        # weight column layout in wk_d (from pack_weights): tap t = kx*3+ky
        # pair stationary for ky: cols of tap (0,ky) | tap (2,ky)  -> need
        # them adjacent. We instead issue the pair matmul with a single AP
        # over a repacked weight tile: host packs wkp with layout
        #   [pair ky=0 (128)] [pair ky=1 (128)] [pair ky=2 (128)]
        #   [single ky=0 (64)] [single ky=1 (64)] [single ky=2 (64)]
        # (see pack_weights_paired). wk_d already holds that layout.

        repl = globals().get("IN_REPL", False)
        slabs = []
        for s in range(N_SLABS):
            L = SLAB_PLANES * PLANE
            xs = xs_pool.tile([KPART, L], mdt)
            if do_in and not repl:
                base = (s * SLAB_OUT) * PLANE
                in_eng.dma_start(
                    out=xs[:, :], in_=xim_d[:, base : base + SLAB_PLANES * PLANE]
                )
            elif do_in and repl:
                base = (s * SLAB_OUT) * PLANE
                # DMA only the dz=0 block into partitions 32-63
                in_eng.dma_start(
                    out=xs[32:64, :], in_=xim_d[:, base : base + L]
                )
                nrows = L // GRID  # z-rows in this slab
                x3 = xs.rearrange("p (r z) -> p r z", z=GRID)
                # block j=0 (dz=-1): dest z 1..31 <- src z 0..30; z=0 col zero
                nc.vector.tensor_copy(x3[0:32, :, 1:GRID], x3[32:64, :, 0 : GRID - 1])
                nc.gpsimd.memset(x3[0:32, :, 0:1], 0.0)
                # block j=2 (dz=+1): dest z 0..30 <- src z 1..31; z=31 col zero
                nc.scalar.copy(x3[64:96, :, 0 : GRID - 1], x3[32:64, :, 1:GRID])
                nc.gpsimd.memset(x3[64:96, :, GRID - 1 : GRID], 0.0)
            else:
                # token write so the tile counts as allocated; reads of the
                # rest are don't-care garbage (timing-only mode)
                nc.sync.dma_start(out=xs[:, 0:128], in_=xim_d[:, 0:128])
            slabs.append(xs)

        def rhs_slice(xp, y):
            """contiguous [96, 512] covering 16 y-rows from padded plane xp,
            y-row start y (0..33)."""
            s = min(max((xp - 0) // SLAB_OUT, 0), N_SLABS - 1)
            # slab s covers padded planes [s*8, s*8+10)
            if xp >= s * SLAB_OUT + SLAB_PLANES:
                s += 1
            if xp < s * SLAB_OUT:
                s -= 1
            loc = xp - s * SLAB_OUT
            off = loc * PLANE + y * GRID
            return slabs[s][:, off : off + 512]

        banks = {}  # (xi, h) -> psum tile
        obs = []

        def evac(xi, h):
            if not do_evac:
                del banks[(xi, h)]
                return
            y0 = h * 16
            ob = ob_pool.tile([DIM_OUT, 512], f32)
            pa = banks[(xi, h)]
            if globals().get("EVAC_SPLIT", True):
                nc.scalar.copy(ob[:, :], pa[0:DIM_OUT, :])
            else:
                nc.vector.tensor_copy(ob[:, :], pa[0:DIM_OUT, :])
            if xi < GRID - 1:
                pb = banks[(xi + 2, h)]
                nc.vector.tensor_add(ob[:, :], ob[:, :], pb[DIM_OUT : 2 * DIM_OUT, :])
            out_eng.dma_start(
                out=out_d[:, xi * 1024 + y0 * GRID : xi * 1024 + y0 * GRID + 512],
                in_=ob[:, :],
            )
            del banks[(xi, h)]

        # groups xi = 0..32; group 32 runs pairs only (feeds out plane 30)
        for xb in range(GRID // 2 + 1):  # blocks of up to 2 plane-groups
            gxs = [g for g in (2 * xb, 2 * xb + 1) if g <= GRID]
            for g in gxs:
                for h in (0, 1):
                    banks[(g, h)] = ps_pool.tile(
                        [2 * DIM_OUT, 512], f32, name=f"bank_{g}_{h}", tag="bank"
                    )
            # weight-major inner order: load each stationary once per block
            for w in range(6):
                for xi in gxs:
                    if xi == GRID and w >= 3:
                        continue  # group 32: pairs only
                    if not do_mm:
                        continue
                    for h in (0, 1):
                        y0 = h * 16
                        ps = banks[(xi, h)]
                        if w < 3:  # pair ky=w, base plane xp=xi
                            ky = 0 if one_weight else w
                            nc.tensor.matmul(
                                out=ps[:, :],
                                lhsT=wk_t[:, 128 * ky : 128 * (ky + 1)],
                                rhs=rhs_slice(xi, y0 + w),
                                start=(w == 0),
                                stop=(w == 5) or (xi == GRID and w == 2),
                            )
                        else:  # single ky=w-3, base plane xp=xi+1
                            ky = 0 if one_weight else w - 3
                            nc.tensor.matmul(
                                out=ps[0:DIM_OUT, :],
                                lhsT=wk_t[:, 384 + 64 * ky : 384 + 64 * (ky + 1)],
                                rhs=rhs_slice(xi + 1, y0 + (w - 3)),
                                start=False,
                                stop=(w == 5),
                            )
            # evacuate planes whose dependencies are now complete:
            # after block xb (groups 2xb, 2xb+1), planes 2xb-2 and 2xb-1 are ready
            for g in gxs:
                xr = g - 2
                if 0 <= xr < GRID:
                    for h in (0, 1):
                        evac(xr, h)
        # tail: plane 31 depends only on its own bank; group-32 banks held
        # rows 64-127 for plane 30 and are dropped now
        for h in (0, 1):
            evac(GRID - 1, h)
            del banks[(GRID, h)]
        assert not banks, f"unevacuated banks: {list(banks)}"


# --------------------------------------------------------------------------
# runner
# --------------------------------------------------------------------------
def _get_program():
    if "nc" not in _PROGRAM_CACHE:
        _PROGRAM_CACHE["nc"] = build_program()
    return _PROGRAM_CACHE["nc"]


def kernel(x, w_lin0, w_lin1, w000, w011, w101, w110):
    from concourse.bass_utils import run_bass_kernel_spmd

    x = np.asarray(x, np.float32)
    k = build_conv_kernel(
        np.asarray(w_lin0), np.asarray(w_lin1),
        np.asarray(w000), np.asarray(w011), np.asarray(w101), np.asarray(w110),
    )
    wk = pack_weights(k)

    in_maps = []
    for b in range(BATCH):
        in_maps.append({"xim": build_im2col(x[b]), "wk": wk})

    nc = _get_program()
    res = run_bass_kernel_spmd(nc, in_maps, list(range(N_CORES)))

    out = np.empty((BATCH, GRID, GRID, GRID, DIM_OUT), np.float32)
    for b in range(BATCH):
        ob = res.results[b]["out"]  # [64, 32768]
        out[b] = ob.reshape(DIM_OUT, GRID, GRID, GRID).transpose(1, 2, 3, 0)
    return out
